# revision 29
# baseline (speedup 1.0000x reference)
"""DeltaNet forward kernel for Trainium2, sharded over 8 NeuronCores.

Sharding: core c handles batch c//2 and head-pair c%2 (heads {2*(c%2), 2*(c%2)+1}).
Host pre-transposes all weights/activations into the layouts the device needs.

v2: all matmul operands bf16 (PSUM accum stays f32), every PE transpose replaced
by an SBUF->SBUF X-bar DMA transpose (knT, Mb, P2T, oT), conv diagonal operands
built on-chip from an 8KB cw DMA, ~4us of PE prewarm matmuls at t=0 so the HAM
clock gate opens before real work, k|beta weight columns DMA'd first, and the
cheap elementwise chains (sigmoid-from-tanh, rsqrt Newton, kbr, S2) moved to the
otherwise-idle GpSimd engine.

Per chunk the work is split into three pipeline stages so the PE always has
S-independent work: stage_a (projections + k-norm + Tinv/W/Mqk), s_advance
(the sequential U/O/S chain), tail (gated rmsnorm + output projection), with
stage_a running 2 chunks ahead of tail.
"""

import sys

for _p in ("/opt/trn_rl_repo", "/root/.axon_site"):
    if _p not in sys.path:
        sys.path.insert(0, _p)

import numpy as np
import ml_dtypes

import concourse.bass as bass
import concourse.tile as tile
from concourse import bacc, mybir
from concourse.bass_utils import run_bass_kernel_spmd
from concourse.masks import make_identity

F32 = mybir.dt.float32
BF16 = mybir.dt.bfloat16
I32 = mybir.dt.int32
NPBF = ml_dtypes.bfloat16

B, L, D, H = 4, 2048, 1024, 4
DK, DV = 512, 1024
HK, HV = 128, 256
CONV, EPS = 4, 1e-5
C = 128            # delta-rule chunk length
NCH = L // C       # 16 chunks
LB = 512           # L-block for conv / q projection
CPB = LB // C      # 4 chunks per L-block
KD = D // 128      # 8 contraction slices
HPC = 2            # heads per core
N_CORES = 8
QSCALE = HK ** -0.5
# wr column layout: [k(256) | beta(2) | v(512) | g(512)]
KB0, KB1 = 0, 258
V0, V1 = 258, 770
G0, G1 = 770, 1282
WRC = 1282
MAGIC = 0x5F3759DF
N_WARM = 4         # prewarm matmuls (~1.7us at cold clock)


def build_program():
    nc = bacc.Bacc(
        "TRN2", target_bir_lowering=False, debug=False,
        enable_asserts=False, num_devices=N_CORES,
    )

    hsT = nc.dram_tensor("hsT", [D, L + 3], BF16, kind="ExternalInput").ap()
    wq = nc.dram_tensor("wq", [D, HPC * HK], BF16, kind="ExternalInput").ap()
    wr = nc.dram_tensor("wr", [D, WRC], BF16, kind="ExternalInput").ap()
    wo = nc.dram_tensor("wo", [HPC * HV, D], BF16, kind="ExternalInput").ap()
    cwt = nc.dram_tensor("cwt", [128, KD * CONV], F32, kind="ExternalInput").ap()
    y = nc.dram_tensor("y", [L, D], F32, kind="ExternalOutput").ap()

    with tile.TileContext(nc) as tc:
        _build_body(nc, tc, hsT, wq, wr, wo, cwt, y)
    nc.compile()
    return nc


def _build_body(nc, tc, hsT, wq, wr, wo, cwt, y):
    from contextlib import ExitStack

    AF = mybir.AluOpType
    ACT = mybir.ActivationFunctionType

    ctx = ExitStack()
    const = ctx.enter_context(tc.tile_pool(name="const", bufs=1))
    # PSUM: 8 banks. psP: projections/conv; psC: chunk (Neumann) math;
    # psS: the serial U/O/S chain + output projection.
    psP = ctx.enter_context(tc.tile_pool(name="psP", bufs=4, space="PSUM"))
    psC = ctx.enter_context(tc.tile_pool(name="psC", bufs=2, space="PSUM"))
    psS = ctx.enter_context(tc.tile_pool(name="psS", bufs=2, space="PSUM"))
    hpool = ctx.enter_context(tc.tile_pool(name="hpool", bufs=2))
    xpool = ctx.enter_context(tc.tile_pool(name="xpool", bufs=2))
    qk = ctx.enter_context(tc.tile_pool(name="qk", bufs=2))
    sS = ctx.enter_context(tc.tile_pool(name="sS", bufs=6))
    ck = ctx.enter_context(tc.tile_pool(name="ck", bufs=10))
    ckx = ctx.enter_context(tc.tile_pool(name="ckx", bufs=12))
    cv = ctx.enter_context(tc.tile_pool(name="cv", bufs=5))
    cu = ctx.enter_context(tc.tile_pool(name="cu", bufs=6))
    scr = ctx.enter_context(tc.tile_pool(name="scr", bufs=4))
    sm = ctx.enter_context(tc.tile_pool(name="sm", bufs=6))

    def cp_act(dst, src):
        nc.scalar.copy(dst, src)

    def cp_dve(dst, src):
        nc.vector.tensor_copy(dst, src)

    cp_state = [0]

    def cp_alt(dst, src):
        cp_state[0] ^= 1
        (cp_act if cp_state[0] else cp_dve)(dst, src)

    # ---- constants (no DMA needed) ----
    identb = const.tile([128, 128], BF16)
    make_identity(nc, identb)
    # warm moving tile: identity in cols 0:128, zeros elsewhere
    wg = const.tile([128, 512], BF16)
    nc.vector.memset(wg, 0.0)
    cp_dve(wg[:, 0:128], identb)
    # umask: 1 where free >= part (upper incl diag); lowm: 1 where free < part
    umask = const.tile([128, 128], F32)
    nc.gpsimd.memset(umask, 1.0)
    nc.gpsimd.affine_select(
        out=umask, in_=umask, compare_op=AF.is_ge, fill=0.0,
        base=0, channel_multiplier=-1, pattern=[[1, 128]],
    )
    lowm = const.tile([128, 128], F32)
    nc.vector.tensor_scalar(lowm, umask, -1.0, 1.0, AF.mult, AF.add)
    magic = const.tile([128, 2], I32)
    nc.vector.memset(magic, MAGIC)
    ones_i = const.tile([128, 1], I32)
    nc.vector.memset(ones_i, 1)

    # ---- PE prewarm: N_WARM N=512 matmuls on constants; the final result
    # (== identity) is used as the Neumann I so the chain stays live.
    psw = psP.tile([128, 512], F32, tag="psP")
    for w in range(N_WARM):
        nc.tensor.matmul(psw, identb, wg, start=True, stop=True)
    wout = const.tile([128, 128], BF16)
    cp_dve(wout, psw[:, 0:128])

    # ---- weights ----
    cws = const.tile([128, KD * CONV], F32)
    nc.sync.dma_start(out=cws, in_=cwt)
    # conv diagonal operands built on-chip: dgs[:, t, :] = diag(cw[:, t])
    dgs = const.tile([128, KD * CONV, 128], BF16)
    for t in range(KD * CONV):
        nc.vector.tensor_scalar_mul(dgs[:, t, :], identb, cws[:, t:t + 1])
    wrs = const.tile([128, KD, WRC], BF16)
    for ks in range(KD):  # k|beta columns first: chunk-0 stage_a needs them
        nc.sync.dma_start(out=wrs[:, ks, KB0:KB1],
                          in_=wr[ks * 128:(ks + 1) * 128, KB0:KB1])
    wqs = const.tile([128, KD, HPC * HK], BF16)
    for ks in range(KD):
        nc.sync.dma_start(out=wqs[:, ks, :], in_=wq[ks * 128:(ks + 1) * 128, :])
    for ks in range(KD):
        nc.sync.dma_start(out=wrs[:, ks, KB1:WRC],
                          in_=wr[ks * 128:(ks + 1) * 128, KB1:WRC])
    wos = const.tile([128, 4, D], BF16)
    for s in range(4):
        nc.sync.dma_start(out=wos[:, s, :], in_=wo[s * 128:(s + 1) * 128, :])

    # ---- state ----
    S = []
    for h in range(HPC):
        st = sS.tile([128, HV], BF16, tag="S")
        nc.vector.memset(st, 0.0)
        S.append(st)

    def rsqrt1(x, n):
        """1/sqrt(x) for x [128, n] f32 SBUF via int bit-trick + 1 Newton."""
        sh = sm.tile([128, n], I32, tag="rs_sh")
        nc.vector.tensor_scalar(
            sh, x.bitcast(I32), ones_i[:, 0:1], None, AF.logical_shift_right
        )
        y0 = sm.tile([128, n], I32, tag="rs_y0")
        nc.vector.tensor_sub(y0, magic[:, 0:n], sh)
        yv = y0.bitcast(F32)
        t = sm.tile([128, n], F32, tag="rs_t")
        nc.vector.tensor_mul(t, yv, yv)
        a = sm.tile([128, n], F32, tag="rs_a")
        nc.vector.scalar_tensor_tensor(
            out=a, in0=x, scalar=-0.5, in1=t, op0=AF.mult, op1=AF.mult
        )
        yn = sm.tile([128, n], F32, tag="rs_y")
        nc.vector.scalar_tensor_tensor(
            out=yn, in0=a, scalar=1.5, in1=yv, op0=AF.add, op1=AF.mult
        )
        return yn

    def stage_lb(lb):
        """Load hsT block, causal conv + silu -> xT, q projection -> qT."""
        hT = hpool.tile([128, KD, LB + 3], BF16, tag="hT")
        for ks in range(KD):
            nc.gpsimd.dma_start(
                out=hT[:, ks, :],
                in_=hsT[ks * 128:(ks + 1) * 128, lb * LB:lb * LB + LB + 3],
            )
        xT = xpool.tile([128, KD, LB], BF16, tag="xT")
        for d in range(KD):
            pc = psP.tile([128, LB], F32, tag="psP")
            for j in range(CONV):
                nc.tensor.matmul(
                    pc, dgs[:, d * CONV + j, :], hT[:, d, j:j + LB],
                    start=(j == 0), stop=(j == CONV - 1),
                )
            nc.scalar.activation(xT[:, d, :], pc, ACT.Silu)
        qT = qk.tile([128, HPC, LB], BF16, tag="qT")
        for h in range(HPC):
            pp = psP.tile([128, LB], F32, tag="psP")
            for ks in range(KD):
                nc.tensor.matmul(
                    pp, wqs[:, ks, h * 128:(h + 1) * 128], xT[:, ks, :],
                    start=(ks == 0), stop=(ks == KD - 1),
                )
            (cp_act if h == 0 else cp_dve)(qT[:, h, :], pp)
        return qT, xT

    def stage_a(c, qT, xT):
        """S-independent chunk work: projections, k-norm, Tinv, W, Mqk."""
        ch = c % CPB
        csl = slice(ch * C, (ch + 1) * C)

        # k/beta projection first: its (serial) norm chain overlaps the
        # v/g projection matmuls that follow.
        pkb = psP.tile([128, KB1], F32, tag="psP")
        for ks in range(KD):
            nc.tensor.matmul(pkb, xT[:, ks, csl], wrs[:, ks, KB0:KB1],
                             start=(ks == 0), stop=(ks == KD - 1))
        # beta = sigmoid(z) = 0.5 + 0.5*tanh(z/2); nbeta = -beta
        th = sm.tile([128, HPC], F32, tag="th")
        nc.scalar.activation(th, pkb[:, 256:258], ACT.Tanh, scale=0.5)
        beta = sm.tile([128, HPC], F32, tag="beta")
        nc.vector.tensor_scalar(beta, th, 0.5, 0.5, AF.mult, AF.add)
        nbeta = sm.tile([128, HPC], F32, tag="nbeta")
        nc.vector.tensor_scalar(nbeta, th, -0.5, -0.5, AF.mult, AF.add)
        # k norms (both heads batched into [128, 2])
        nsq = sm.tile([128, HPC], F32, tag="nsq")
        for h in range(HPC):
            sq = scr.tile([128, 128], BF16, tag="sq")
            nc.scalar.activation(
                sq, pkb[:, h * 128:(h + 1) * 128], ACT.Square,
                accum_out=nsq[:, h:h + 1],
            )
        invc = rsqrt1(nsq, HPC)

        pv = psP.tile([128, HPC * HV], F32, tag="psP")
        pg = psP.tile([128, HPC * HV], F32, tag="psP")
        for ks in range(KD):
            lx = xT[:, ks, csl]
            nc.tensor.matmul(pv, lx, wrs[:, ks, V0:V1],
                             start=(ks == 0), stop=(ks == KD - 1))
            nc.tensor.matmul(pg, lx, wrs[:, ks, G0:G1],
                             start=(ks == 0), stop=(ks == KD - 1))
        sg = cv.tile([128, HPC * HV], BF16, tag="sg")
        nc.scalar.activation(sg, pg, ACT.Silu)
        vb = cv.tile([128, HPC * HV], BF16, tag="vb")
        for h in range(HPC):
            hsl = slice(h * HV, (h + 1) * HV)
            nc.vector.tensor_scalar_mul(vb[:, hsl], pv[:, hsl], beta[:, h:h + 1])

        art = {"vb": vb, "sg": sg, "qT": qT, "csl": csl, "h": []}
        for h in range(HPC):
            knr = ckx.tile([128, 128], BF16, tag="knr")
            nc.vector.tensor_scalar_mul(
                knr, pkb[:, h * 128:(h + 1) * 128], invc[:, h:h + 1]
            )
            ptk = psC.tile([128, 128], BF16, tag="psC")
            nc.tensor.transpose(ptk, knr, identb)
            knT = ck.tile([128, 128], BF16, tag="knT")
            cp_act(knT, ptk)

            # G = Kn Kn^T; Nb = strict_lower(-beta_i G) = M
            pG = psC.tile([128, 128], F32, tag="psC")
            nc.tensor.matmul(pG, knT, knT, start=True, stop=True)
            Nb = ck.tile([128, 128], BF16, tag="Nb")
            nc.vector.scalar_tensor_tensor(
                out=Nb, in0=pG, scalar=nbeta[:, h:h + 1], in1=lowm,
                op0=AF.mult, op1=AF.mult,
            )
            ptm = psC.tile([128, 128], BF16, tag="psC")
            nc.tensor.transpose(ptm, Nb, identb)
            Mb = ck.tile([128, 128], BF16, tag="Mb")     # M^T
            cp_act(Mb, ptm)

            # Mqk^T = masked Kn Q^T
            pM = psC.tile([128, 128], F32, tag="psC")
            nc.tensor.matmul(pM, knT, qT[:, h, csl], start=True, stop=True)
            mqk = ckx.tile([128, 128], BF16, tag="mqk")
            nc.vector.tensor_mul(mqk, pM, umask)

            # tinvT = sum_{k<8} (M^T)^k via doubling
            S2 = ck.tile([128, 128], BF16, tag="S2")
            nc.gpsimd.tensor_add(S2, Mb, wout)

            pP2 = psC.tile([128, 128], F32, tag="psC")
            nc.tensor.matmul(pP2, Nb, Mb, start=True, stop=True)  # A^2
            P2 = ck.tile([128, 128], BF16, tag="P2")
            cp_alt(P2, pP2)
            pP2T = psC.tile([128, 128], F32, tag="psC")
            nc.tensor.matmul(pP2T, Mb, Nb, start=True, stop=True)  # M^2
            P2T = ck.tile([128, 128], BF16, tag="P2T")
            cp_alt(P2T, pP2T)

            pS4 = psC.tile([128, 128], F32, tag="psC")
            nc.tensor.matmul(pS4, P2T, S2, start=True, stop=True)
            S4 = ck.tile([128, 128], BF16, tag="S4")
            nc.vector.tensor_add(S4, S2, pS4)

            pP4 = psC.tile([128, 128], F32, tag="psC")
            nc.tensor.matmul(pP4, P2, P2T, start=True, stop=True)  # M^4
            P4T = ck.tile([128, 128], BF16, tag="P4T")
            cp_alt(P4T, pP4)

            pTi = psC.tile([128, 128], F32, tag="psC")
            nc.tensor.matmul(pTi, P4T, S4, start=True, stop=True)
            tinvT = ckx.tile([128, 128], BF16, tag="tinvT")
            nc.vector.tensor_add(tinvT, S4, pTi)
            art["h"].append({"knr": knr, "knT": knT, "mqk": mqk,
                             "tinvT": tinvT, "nbeta": nbeta})
        return art

    def s_advance(c, art):
        """Sequential S-chain: U, O (matmuls only), S update."""
        vb, qT, csl = art["vb"], art["qT"], art["csl"]
        art["O"] = []
        ms = sm.tile([128, HPC], F32, tag="ms")
        art["ms"] = ms
        for h in range(HPC):
            a = art["h"][h]
            hsl = slice(h * HV, (h + 1) * HV)
            # U = TinvT.T @ (vb - beta * Kn S)
            pkS = psS.tile([128, HV], F32, tag="psS")
            nc.tensor.matmul(pkS, a["knT"], S[h], start=True, stop=True)
            UV = cu.tile([128, HV], BF16, tag="UV")
            nc.vector.scalar_tensor_tensor(
                out=UV, in0=pkS, scalar=a["nbeta"][:, h:h + 1], in1=vb[:, hsl],
                op0=AF.mult, op1=AF.add,
            )
            pU = psS.tile([128, HV], F32, tag="psS")
            nc.tensor.matmul(pU, a["tinvT"], UV, start=True, stop=True)
            U = cu.tile([128, HV], BF16, tag="U")
            cp_act(U, pU)

            pO = psS.tile([128, HV], F32, tag="psS")
            nc.tensor.matmul(pO, qT[:, h, csl], S[h], start=True, stop=False)
            nc.tensor.matmul(pO, a["mqk"], U, start=False, stop=True)
            O_s = cu.tile([128, HV], BF16, tag="O")
            cp_dve(O_s, pO)
            sq2 = scr.tile([128, HV], BF16, tag="sq2")
            nc.scalar.activation(sq2, O_s, ACT.Square, accum_out=ms[:, h:h + 1])
            art["O"].append(O_s)

            pD = psS.tile([128, HV], F32, tag="psS")
            nc.tensor.matmul(pD, a["knr"], U, start=True, stop=True)
            Sn = sS.tile([128, HV], BF16, tag="S")
            nc.vector.tensor_add(Sn, S[h], pD)
            S[h] = Sn

    def tail(c, art):
        """Gated rmsnorm + output projection + store."""
        sg = art["sg"]
        msb = sm.tile([128, HPC], F32, tag="msb")
        nc.vector.tensor_scalar(msb, art["ms"], 1.0 / HV, EPS, AF.mult, AF.add)
        rs = rsqrt1(msb, HPC)
        ofin = cv.tile([128, HPC * HV], BF16, tag="ofin")
        for h in range(HPC):
            hsl = slice(h * HV, (h + 1) * HV)
            nc.vector.scalar_tensor_tensor(
                out=ofin[:, hsl], in0=art["O"][h], scalar=rs[:, h:h + 1],
                in1=sg[:, hsl], op0=AF.mult, op1=AF.mult,
            )
        oT = ckx.tile([128, 4, 128], BF16, tag="oT")
        nc.sync.dma_start_transpose(oT, ofin)
        for t2 in range(2):
            py = psS.tile([128, 512], F32, tag="psS")
            for s in range(4):
                nc.tensor.matmul(
                    py, oT[:, s, :], wos[:, s, t2 * 512:(t2 + 1) * 512],
                    start=(s == 0), stop=(s == 3),
                )
            yst = cv.tile([128, 512], F32, tag="yst")
            (cp_act if t2 == 0 else cp_dve)(yst, py)
            nc.sync.dma_start(
                out=y[c * 128:(c + 1) * 128, t2 * 512:(t2 + 1) * 512], in_=yst
            )

    # software pipeline: s_advance(c-2) | tail(c-4) | stage_a(c).
    # The serial S-chain goes first in program order so the scheduler
    # prioritizes it; stage_a fills the pipeline for later chunks.
    arts = {}
    cur = stage_lb(0)
    nxt = None
    for t in range(NCH + 4):
        if 2 <= t < NCH + 2:
            s_advance(t - 2, arts[t - 2])
        if t >= 4:
            tail(t - 4, arts.pop(t - 4))
        if t < NCH:
            if t % CPB == 0 and t > 0:
                cur = nxt
            arts[t] = stage_a(t, *cur)
            # prefetch the next L-block's conv + q projection 2 chunks early
            if t % CPB == CPB - 2 and t + 2 < NCH:
                nxt = stage_lb((t + 2) // CPB)

    ctx.close()


_nc_cache = None


def _get_nc():
    global _nc_cache
    if _nc_cache is None:
        _nc_cache = build_program()
    return _nc_cache


def make_in_maps(hidden_states, conv_w, Wq, Wk, Wv, Wb, Wg, Wo, rms_weight):
    f32 = lambda a: np.asarray(a, dtype=np.float32)
    hs, cw = f32(hidden_states), f32(conv_w)
    Wq, Wk, Wv, Wb, Wg, Wo, rmsw = (
        f32(Wq), f32(Wk), f32(Wv), f32(Wb), f32(Wg), f32(Wo), f32(rms_weight)
    )
    bf = lambda a: np.ascontiguousarray(a).astype(NPBF)

    # conv weights as [128, KD*CONV]: cwt[p, d*CONV+j] = cw[d*128+p, j]
    cwt = np.empty((128, KD * CONV), np.float32)
    for d in range(KD):
        for j in range(CONV):
            cwt[:, d * CONV + j] = cw[d * 128:(d + 1) * 128, j]

    rms2 = np.tile(rmsw, HPC)[:, None]  # [512, 1]
    in_maps = []
    for core in range(N_CORES):
        b, g = core // 2, core % 2
        hsT = np.zeros((D, L + 3), np.float32)
        hsT[:, 3:] = hs[b].T
        wrcat = np.concatenate(
            [
                Wk[g * HPC * HK:(g + 1) * HPC * HK].T,
                Wb[g * HPC:(g + 1) * HPC].T,
                Wv[g * HPC * HV:(g + 1) * HPC * HV].T,
                Wg[g * HPC * HV:(g + 1) * HPC * HV].T,
            ],
            axis=1,
        )  # [D, 1282]  (k|beta|v|g)
        in_maps.append({
            "hsT": bf(hsT),
            "wq": bf(Wq[g * HPC * HK:(g + 1) * HPC * HK].T * QSCALE),
            "wr": bf(wrcat),
            "wo": bf(Wo[:, g * HPC * HV:(g + 1) * HPC * HV].T * rms2),
            "cwt": cwt,
        })
    return in_maps


def unshard(results):
    y = np.empty((B, L, D), np.float32)
    for b in range(B):
        y[b] = results[2 * b]["y"] + results[2 * b + 1]["y"]
    return y


def kernel(hidden_states, conv_w, Wq, Wk, Wv, Wb, Wg, Wo, rms_weight, **_ignored):
    nc = _get_nc()
    in_maps = make_in_maps(hidden_states, conv_w, Wq, Wk, Wv, Wb, Wg, Wo, rms_weight)
    res = run_bass_kernel_spmd(nc, in_maps, core_ids=list(range(N_CORES)))
    return unshard(res.results)
